# revision 10
# baseline (speedup 1.0000x reference)
"""Trainium2 Bass kernel for nn_PerformerAttention.

reference math (B,H,S,D = 4,8,2048,64):
    qf = relu(q @ W.T); kf = relu(k @ W.T)          # [B,H,S,D]
    scores = qf @ kf.T                              # [B,H,S,S]
    attn_weights = softmax(scores, axis=-1)
    attn_output  = v * rowsum(attn_weights) == v    # softmax rows sum to 1
    returns (attn_output, attn_weights)

Sharding: B*H = 32 (b,h) pairs, 4 per core across 8 cores.  Host-side
layout prep only: q/k transposed to [.., D, S] (matmul contracts over
the partition dim, so the device needs no transposes) and split into
bf16 hi/lo pairs (hi + lo == fp32 value to ~2^-17).

All matmuls run as 3-term bf16 splits (a@b = ahi@bhi + ahi@blo +
alo@bhi; products are exact in the PE, only the lo@lo term is dropped,
~1e-4..1e-3 rel err) because fp32 matmul costs 2 half-rate passes on
TRN2 (~2x the cost model).  Scores per 128-row tile accumulate in a
[128, S] fp32 PSUM tile; softmax is:
    VectorE: fused PSUM->SBUF copy(-scores) + row-max accumulator
    ScalarE: exp(scale*x+bias) with fused row-sum
    normalize-mul on V or S per cfg pattern; DMA out.
Feature transforms for the next head are emitted one 512-col chunk at
a time between score tiles so they hide inside the V/S-bound pipeline
slack instead of stalling it.
"""

import os
import numpy as np

B, H, S, D = 4, 8, 2048, 64
NCORES = 8
HPC = (B * H) // NCORES      # heads per core = 4
PAIRS = HPC // 2             # head pairs per core = 2
RT = S // 128                # 128-row score tiles per head = 16
NCH = S // 512               # 512-col matmul chunks per row tile = 4

# Tunables (test.py may override before calling kernel()).
CONFIG = {
    "copymax": True,      # fused PSUM->SBUF copy + row-max on VectorE
    "mul_pattern": "vvs", # normalize-mul engine per tile, cycled: v/s/g
    "hiprio": True,       # schedule PSUM-releasing ops ahead of lagging muls
    "trace": False,       # request NTFF profile from the run
}

_CACHE = {}


def _build_program(cfg):
    from contextlib import ExitStack, nullcontext

    import concourse.bacc as bacc
    import concourse.mybir as mybir
    import concourse.tile as tile

    f32 = mybir.dt.float32
    bf16 = mybir.dt.bfloat16
    AF = mybir.ActivationFunctionType
    OP = mybir.AluOpType
    AX = mybir.AxisListType

    nc = bacc.Bacc(
        "TRN2",
        target_bir_lowering=False,
        debug=False,
        num_devices=NCORES,
    )

    ins = {}
    for nm in ("qthi", "qtlo", "kthi", "ktlo"):
        ins[nm] = nc.dram_tensor(nm, [PAIRS * 128, S], bf16, kind="ExternalInput").ap()
    for nm in ("wthi", "wtlo"):
        ins[nm] = nc.dram_tensor(nm, [128, D], bf16, kind="ExternalInput").ap()
    out = nc.dram_tensor("out", [HPC * S, S], f32, kind="ExternalOutput").ap()

    with tile.TileContext(nc) as tc, ExitStack() as ctx:
        const = ctx.enter_context(tc.tile_pool(name="const", bufs=1))
        inp = ctx.enter_context(tc.tile_pool(name="inp", bufs=2))
        feat = ctx.enter_context(tc.tile_pool(name="feat", bufs=2))
        psum = ctx.enter_context(tc.tile_pool(name="psum", bufs=2, space="PSUM"))
        work = ctx.enter_context(tc.tile_pool(name="work", bufs=5))
        stat = ctx.enter_context(tc.tile_pool(name="stat", bufs=8))
        outp = ctx.enter_context(tc.tile_pool(name="outp", bufs=5))

        wh = const.tile([128, D], bf16, tag="wh", name="wh")
        nc.sync.dma_start(wh[:], ins["wthi"][:, :])
        wl = const.tile([128, D], bf16, tag="wl", name="wl")
        nc.sync.dma_start(wl[:], ins["wtlo"][:, :])

        tile_idx = 0
        hp = (lambda: tc.high_priority()) if cfg["hiprio"] else (lambda: nullcontext())

        def softmax_tail(ps, h, m):
            """PSUM scores tile -> normalized SBUF tile -> DMA out."""
            nonlocal tile_idx
            negmax = stat.tile([128, 1], f32, tag="negmax", name="negmax")
            rowsum = stat.tile([128, 1], f32, tag="rowsum", name="rowsum")
            rinv = stat.tile([128, 1], f32, tag="rinv", name="rinv")
            expt = work.tile([128, S], f32, tag="expt", name="expt")
            if cfg["copymax"]:
                # sc = -scores (SBUF copy), negmax = min(-scores) = -rowmax.
                # Frees the PSUM tile after this single VectorE pass.  High
                # priority: prefer these over earlier tiles' normalize-muls
                # or the PSUM slots starve the PE.
                sc = work.tile([128, S], f32, tag="sc", name="sc")
                with hp():
                    nc.vector.tensor_scalar(
                        sc[:], ps[:], -1.0, None, OP.mult, OP.min, accum_out=negmax[:]
                    )
                    nc.scalar.activation(
                        expt[:], sc[:], AF.Exp,
                        bias=negmax[:], scale=-1.0, accum_out=rowsum[:],
                    )
            else:
                with hp():
                    nc.vector.reduce_max(negmax[:], ps[:], AX.X, negate=True)
                    nc.scalar.activation(
                        expt[:], ps[:], AF.Exp,
                        bias=negmax[:], scale=1.0, accum_out=rowsum[:],
                    )
            nc.vector.reciprocal(rinv[:], rowsum[:])
            ot = outp.tile([128, S], f32, tag="ot", name="ot")
            eng = cfg["mul_pattern"][tile_idx % len(cfg["mul_pattern"])]
            if eng == "s":
                nc.scalar.activation(ot[:], expt[:], AF.Copy, bias=0.0, scale=rinv[:])
            else:
                nc.vector.tensor_scalar(ot[:], expt[:], rinv[:], None, OP.mult, OP.bypass)
            nc.sync.dma_start(out[h * S + 128 * m : h * S + 128 * (m + 1), :], ot[:])
            tile_idx += 1

        # ---- inputs + features ------------------------------------------
        pair_tiles = {}

        def load_pair(p):
            t = {}
            for nm in ("qthi", "qtlo", "kthi", "ktlo"):
                t[nm] = inp.tile([128, S], bf16, tag=nm, name=nm)
                nc.sync.dma_start(t[nm][:], ins[nm][128 * p : 128 * (p + 1), :])
            pair_tiles[p] = t

        feats = {}  # h -> {"q": (hi, lo), "k": (hi, lo)}

        def alloc_feats(h):
            st = {}
            for w_ in ("q", "k"):
                hi = feat.tile([128, S], bf16, tag=w_ + "hi", name=w_ + "hi")
                lo = feat.tile([128, S], bf16, tag=w_ + "lo", name=w_ + "lo")
                st[w_] = (hi, lo)
            feats[h] = st

        def emit_feature_chunk(h, which, j):
            """One 512-col chunk of the feature transform for head h:
            relu(W^T.T @ xT) into bf16 hi/lo, duplicated across both
            partition halves via PE column tiling."""
            p, e = h // 2, h % 2
            rb = 64 * e
            src_hi = pair_tiles[p]["qthi" if which == "q" else "kthi"]
            src_lo = pair_tiles[p]["qtlo" if which == "q" else "ktlo"]
            hi, lo = feats[h][which]
            cs = slice(512 * j, 512 * (j + 1))
            pf = psum.tile([128, 512], f32, tag="ps", name="pf")
            for c in (0, 64):
                for t, (wmat, smat) in enumerate(
                    ((wh, src_hi), (wh, src_lo), (wl, src_hi))
                ):
                    nc.tensor.matmul(
                        pf[c : c + 64, :],
                        lhsT=wmat[rb : rb + 64, :],
                        rhs=smat[rb : rb + 64, cs],
                        start=(t == 0), stop=(t == 2),
                        tile_position=(rb, c),
                    )
            nc.scalar.activation(hi[:, cs], pf[:, :], AF.Relu)
            # lo = relu(pf) - hi, rounded to bf16 (the dropped residual).
            nc.vector.scalar_tensor_tensor(
                lo[:, cs], pf[:, :], 0.0, hi[:, cs], OP.max, OP.subtract
            )
            del pf

        def score_tile(h, m):
            # The K=64 matmul streams at half rate (1 col / 1.2GHz cycle);
            # row-packing two matmuls on opposite PE halves restores full
            # rate.  Pack the tile's left column half (rows 0-63) against
            # its right half (rows 64-127) — features are duplicated across
            # both partition halves, and both write the same PSUM tile, so
            # serial-tile PSUM pipelining is preserved.
            qhi, qlo = feats[h]["q"]
            khi, klo = feats[h]["k"]
            mc = slice(128 * m, 128 * (m + 1))
            ps = psum.tile([128, S], f32, tag="ps", name="ps")
            for jj in range(NCH // 2):
                for t, (ql, kl) in enumerate(((qhi, khi), (qhi, klo), (qlo, khi))):
                    for half, rb in ((0, 0), (1, 64)):
                        j = jj + half * (NCH // 2)
                        cs = slice(512 * j, 512 * (j + 1))
                        nc.tensor.matmul(
                            ps[:, cs],
                            lhsT=ql[rb : rb + 64, mc],
                            rhs=kl[rb : rb + 64, cs],
                            start=(t == 0), stop=(t == 2),
                            tile_position=(rb, 0),
                        )
            return ps

        # ---- main schedule ----------------------------------------------
        load_pair(0)
        alloc_feats(0)
        for w_ in ("q", "k"):
            for j in range(NCH):
                emit_feature_chunk(0, w_, j)

        for h in range(HPC):
            nxt = h + 1
            # prefetch plan: tile index -> closure
            plan = {}
            if nxt < HPC:
                if nxt % 2 == 0:
                    plan[0] = lambda p=nxt // 2: load_pair(p)
                plan[1] = lambda n=nxt: alloc_feats(n)
                k_ = 2
                for w_ in ("q", "k"):
                    for j in range(NCH):
                        plan[k_] = (
                            lambda n=nxt, ww=w_, jj=j: emit_feature_chunk(n, ww, jj)
                        )
                        k_ += 1
            for m in range(RT):
                ps = score_tile(h, m)
                softmax_tail(ps, h, m)
                act = plan.get(m)
                if act:
                    act()

    nc.compile()
    return nc


def _cfg_key(cfg):
    return (cfg["copymax"], cfg["mul_pattern"], cfg["hiprio"])


def _get_program(cfg):
    key = _cfg_key(cfg)
    if key not in _CACHE:
        _CACHE[key] = _build_program(cfg)
    return _CACHE[key]


def _split_bf16(x):
    import ml_dtypes

    hi = x.astype(ml_dtypes.bfloat16)
    lo = (x - hi.astype(np.float32)).astype(ml_dtypes.bfloat16)
    return hi, lo


def make_in_maps(q, k, random_weights):
    """Host-side sharding/layout prep -> per-core input dicts."""
    q = np.asarray(q, dtype=np.float32)
    k = np.asarray(k, dtype=np.float32)
    w = np.asarray(random_weights, dtype=np.float32)
    # [B,H,S,D] -> [B*H, D, S]
    qT = np.ascontiguousarray(q.transpose(0, 1, 3, 2)).reshape(B * H, D, S)
    kT = np.ascontiguousarray(k.transpose(0, 1, 3, 2)).reshape(B * H, D, S)
    wt = np.concatenate([w.T, w.T], axis=0)  # [128, D] duplicated halves
    wthi, wtlo = _split_bf16(np.ascontiguousarray(wt))
    qhi, qlo = _split_bf16(qT)
    khi, klo = _split_bf16(kT)
    in_maps = []
    for c in range(NCORES):
        sel = slice(HPC * c, HPC * (c + 1))
        in_maps.append({
            "qthi": np.ascontiguousarray(qhi[sel]).reshape(PAIRS * 128, S),
            "qtlo": np.ascontiguousarray(qlo[sel]).reshape(PAIRS * 128, S),
            "kthi": np.ascontiguousarray(khi[sel]).reshape(PAIRS * 128, S),
            "ktlo": np.ascontiguousarray(klo[sel]).reshape(PAIRS * 128, S),
            "wthi": wthi,
            "wtlo": wtlo,
        })
    return in_maps


def run_device(q, k, random_weights, cfg=None, trace=None):
    """Compile (cached), run on all 8 cores, return (attn_weights, results)."""
    from concourse.bass_utils import run_bass_kernel_spmd

    cfg = dict(CONFIG if cfg is None else cfg)
    if trace is not None:
        cfg["trace"] = trace
    nc = _get_program(cfg)
    in_maps = make_in_maps(q, k, random_weights)
    res = run_bass_kernel_spmd(
        nc, in_maps, core_ids=list(range(NCORES)), trace=cfg["trace"]
    )
    outs = [res.results[c]["out"].reshape(HPC, S, S) for c in range(NCORES)]
    attn_weights = np.concatenate(outs, axis=0).reshape(B, H, S, S)
    return attn_weights, res


def kernel(q, k, v, random_weights):
    attn_weights, _ = run_device(q, k, random_weights)
    attn_output = np.asarray(v, dtype=np.float32)
    return attn_output, attn_weights


# revision 11
# speedup vs baseline: 1.0078x; 1.0078x over previous
"""Trainium2 Bass kernel for nn_PerformerAttention.

reference math (B,H,S,D = 4,8,2048,64):
    qf = relu(q @ W.T); kf = relu(k @ W.T)          # [B,H,S,D]
    scores = qf @ kf.T                              # [B,H,S,S]
    attn_weights = softmax(scores, axis=-1)
    attn_output  = v * rowsum(attn_weights) == v    # softmax rows sum to 1
    returns (attn_output, attn_weights)

Sharding: B*H = 32 (b,h) pairs, 4 per core across 8 cores.  Host-side
layout prep only: q/k transposed to [.., D, S] (matmul contracts over
the partition dim, so the device needs no transposes) and split into
bf16 hi/lo pairs (hi + lo == fp32 value to ~2^-17).

All matmuls run as 3-term bf16 splits (a@b = ahi@bhi + ahi@blo +
alo@bhi; products are exact in the PE, only the lo@lo term is dropped,
~1e-4..1e-3 rel err) because fp32 matmul costs 2 half-rate passes on
TRN2 (~2x the cost model).  Scores per 128-row tile accumulate in a
[128, S] fp32 PSUM tile; softmax is:
    VectorE: fused PSUM->SBUF copy(-scores) + row-max accumulator
    ScalarE: exp(scale*x+bias) with fused row-sum
    normalize-mul on V or S per cfg pattern; DMA out.
Feature transforms for the next head are emitted one 512-col chunk at
a time between score tiles so they hide inside the V/S-bound pipeline
slack instead of stalling it.
"""

import os
import numpy as np

B, H, S, D = 4, 8, 2048, 64
NCORES = 8
HPC = (B * H) // NCORES      # heads per core = 4
PAIRS = HPC // 2             # head pairs per core = 2
RT = S // 128                # 128-row score tiles per head = 16
NCH = S // 512               # 512-col matmul chunks per row tile = 4

# Tunables (test.py may override before calling kernel()).
CONFIG = {
    "copymax": True,      # fused PSUM->SBUF copy + row-max on VectorE
    "mul_pattern": "vvs", # normalize-mul engine per tile, cycled: v/s/g
    "hiprio": True,       # schedule PSUM-releasing ops ahead of lagging muls
    "trace": False,       # request NTFF profile from the run
}

_CACHE = {}


def _build_program(cfg):
    from contextlib import ExitStack, nullcontext

    import concourse.bacc as bacc
    import concourse.mybir as mybir
    import concourse.tile as tile

    f32 = mybir.dt.float32
    bf16 = mybir.dt.bfloat16
    AF = mybir.ActivationFunctionType
    OP = mybir.AluOpType
    AX = mybir.AxisListType

    nc = bacc.Bacc(
        "TRN2",
        target_bir_lowering=False,
        debug=False,
        num_devices=NCORES,
    )

    ins = {}
    for nm in ("qthi", "qtlo", "kthi", "ktlo"):
        ins[nm] = nc.dram_tensor(nm, [PAIRS * 128, S], bf16, kind="ExternalInput").ap()
    for nm in ("wthi", "wtlo"):
        ins[nm] = nc.dram_tensor(nm, [128, D], bf16, kind="ExternalInput").ap()
    out = nc.dram_tensor("out", [HPC * S, S], f32, kind="ExternalOutput").ap()

    with tile.TileContext(nc) as tc, ExitStack() as ctx:
        const = ctx.enter_context(tc.tile_pool(name="const", bufs=1))
        inp = ctx.enter_context(tc.tile_pool(name="inp", bufs=2))
        feat = ctx.enter_context(tc.tile_pool(name="feat", bufs=2))
        psum = ctx.enter_context(tc.tile_pool(name="psum", bufs=2, space="PSUM"))
        work = ctx.enter_context(tc.tile_pool(name="work", bufs=5))
        stat = ctx.enter_context(tc.tile_pool(name="stat", bufs=8))
        outp = ctx.enter_context(tc.tile_pool(name="outp", bufs=5))

        wh = const.tile([128, D], bf16, tag="wh", name="wh")
        nc.sync.dma_start(wh[:], ins["wthi"][:, :])
        wl = const.tile([128, D], bf16, tag="wl", name="wl")
        nc.sync.dma_start(wl[:], ins["wtlo"][:, :])

        tile_idx = 0
        hp = (lambda: tc.high_priority()) if cfg["hiprio"] else (lambda: nullcontext())

        pending = []  # deferred (rowsum, expt, h, m) normalize+store work

        def flush_tail():
            """Emit recip+mul+DMA for the oldest pending tile.  Deferred one
            tile so the VectorE stream never blocks on ScalarE's row-sum."""
            nonlocal tile_idx
            if not pending:
                return
            rowsum, expt, h, m = pending.pop(0)
            rinv = stat.tile([128, 1], f32, tag="rinv", name="rinv")
            nc.vector.reciprocal(rinv[:], rowsum[:])
            ot = outp.tile([128, S], f32, tag="ot", name="ot")
            eng = cfg["mul_pattern"][tile_idx % len(cfg["mul_pattern"])]
            if eng == "s":
                nc.scalar.activation(ot[:], expt[:], AF.Copy, bias=0.0, scale=rinv[:])
            else:
                nc.vector.tensor_scalar(ot[:], expt[:], rinv[:], None, OP.mult, OP.bypass)
            nc.sync.dma_start(out[h * S + 128 * m : h * S + 128 * (m + 1), :], ot[:])
            tile_idx += 1

        def softmax_tail(ps, h, m):
            """PSUM scores tile -> exp'd SBUF tile (normalize deferred)."""
            negmax = stat.tile([128, 1], f32, tag="negmax", name="negmax")
            rowsum = stat.tile([128, 1], f32, tag="rowsum", name="rowsum")
            expt = work.tile([128, S], f32, tag="expt", name="expt")
            if cfg["copymax"]:
                # sc = -scores (SBUF copy), negmax = min(-scores) = -rowmax.
                # Frees the PSUM tile after this single VectorE pass.  High
                # priority: prefer these over earlier tiles' normalize-muls
                # or the PSUM slots starve the PE.
                sc = work.tile([128, S], f32, tag="sc", name="sc")
                with hp():
                    nc.vector.tensor_scalar(
                        sc[:], ps[:], -1.0, None, OP.mult, OP.min, accum_out=negmax[:]
                    )
                    nc.scalar.activation(
                        expt[:], sc[:], AF.Exp,
                        bias=negmax[:], scale=-1.0, accum_out=rowsum[:],
                    )
            else:
                with hp():
                    nc.vector.reduce_max(negmax[:], ps[:], AX.X, negate=True)
                    nc.scalar.activation(
                        expt[:], ps[:], AF.Exp,
                        bias=negmax[:], scale=1.0, accum_out=rowsum[:],
                    )
            pending.append((rowsum, expt, h, m))
            if len(pending) > 1:
                flush_tail()

        # ---- inputs + features ------------------------------------------
        pair_tiles = {}

        def load_pair(p):
            t = {}
            for nm in ("qthi", "qtlo", "kthi", "ktlo"):
                t[nm] = inp.tile([128, S], bf16, tag=nm, name=nm)
                nc.sync.dma_start(t[nm][:], ins[nm][128 * p : 128 * (p + 1), :])
            pair_tiles[p] = t

        feats = {}  # h -> {"q": (hi, lo), "k": (hi, lo)}

        def alloc_feats(h):
            st = {}
            for w_ in ("q", "k"):
                hi = feat.tile([128, S], bf16, tag=w_ + "hi", name=w_ + "hi")
                lo = feat.tile([128, S], bf16, tag=w_ + "lo", name=w_ + "lo")
                st[w_] = (hi, lo)
            feats[h] = st

        def emit_feature_chunk(h, which, j):
            """One 512-col chunk of the feature transform for head h:
            relu(W^T.T @ xT) into bf16 hi/lo, duplicated across both
            partition halves via PE column tiling."""
            p, e = h // 2, h % 2
            rb = 64 * e
            src_hi = pair_tiles[p]["qthi" if which == "q" else "kthi"]
            src_lo = pair_tiles[p]["qtlo" if which == "q" else "ktlo"]
            hi, lo = feats[h][which]
            cs = slice(512 * j, 512 * (j + 1))
            pf = psum.tile([128, 512], f32, tag="ps", name="pf")
            for c in (0, 64):
                for t, (wmat, smat) in enumerate(
                    ((wh, src_hi), (wh, src_lo), (wl, src_hi))
                ):
                    nc.tensor.matmul(
                        pf[c : c + 64, :],
                        lhsT=wmat[rb : rb + 64, :],
                        rhs=smat[rb : rb + 64, cs],
                        start=(t == 0), stop=(t == 2),
                        tile_position=(rb, c),
                    )
            nc.scalar.activation(hi[:, cs], pf[:, :], AF.Relu)
            # lo = relu(pf) - hi, rounded to bf16 (the dropped residual).
            nc.vector.scalar_tensor_tensor(
                lo[:, cs], pf[:, :], 0.0, hi[:, cs], OP.max, OP.subtract
            )
            del pf

        def score_tile(h, m):
            # The K=64 matmul streams at half rate (1 col / 1.2GHz cycle);
            # row-packing two matmuls on opposite PE halves restores full
            # rate.  Pack the tile's left column half (rows 0-63) against
            # its right half (rows 64-127) — features are duplicated across
            # both partition halves, and both write the same PSUM tile, so
            # serial-tile PSUM pipelining is preserved.
            qhi, qlo = feats[h]["q"]
            khi, klo = feats[h]["k"]
            mc = slice(128 * m, 128 * (m + 1))
            ps = psum.tile([128, S], f32, tag="ps", name="ps")
            for jj in range(NCH // 2):
                for t, (ql, kl) in enumerate(((qhi, khi), (qhi, klo), (qlo, khi))):
                    for half, rb in ((0, 0), (1, 64)):
                        j = jj + half * (NCH // 2)
                        cs = slice(512 * j, 512 * (j + 1))
                        nc.tensor.matmul(
                            ps[:, cs],
                            lhsT=ql[rb : rb + 64, mc],
                            rhs=kl[rb : rb + 64, cs],
                            start=(t == 0), stop=(t == 2),
                            tile_position=(rb, 0),
                        )
            return ps

        # ---- main schedule ----------------------------------------------
        load_pair(0)
        alloc_feats(0)
        for w_ in ("q", "k"):
            for j in range(NCH):
                emit_feature_chunk(0, w_, j)

        for h in range(HPC):
            nxt = h + 1
            # prefetch plan: tile index -> closure
            plan = {}
            if nxt < HPC:
                if nxt % 2 == 0:
                    plan[0] = lambda p=nxt // 2: load_pair(p)
                plan[1] = lambda n=nxt: alloc_feats(n)
                k_ = 2
                for w_ in ("q", "k"):
                    for j in range(NCH):
                        plan[k_] = (
                            lambda n=nxt, ww=w_, jj=j: emit_feature_chunk(n, ww, jj)
                        )
                        k_ += 1
            for m in range(RT):
                ps = score_tile(h, m)
                softmax_tail(ps, h, m)
                act = plan.get(m)
                if act:
                    act()
        while pending:
            flush_tail()

    nc.compile()
    return nc


def _cfg_key(cfg):
    return (cfg["copymax"], cfg["mul_pattern"], cfg["hiprio"])


def _get_program(cfg):
    key = _cfg_key(cfg)
    if key not in _CACHE:
        _CACHE[key] = _build_program(cfg)
    return _CACHE[key]


def _split_bf16(x):
    import ml_dtypes

    hi = x.astype(ml_dtypes.bfloat16)
    lo = (x - hi.astype(np.float32)).astype(ml_dtypes.bfloat16)
    return hi, lo


def make_in_maps(q, k, random_weights):
    """Host-side sharding/layout prep -> per-core input dicts."""
    q = np.asarray(q, dtype=np.float32)
    k = np.asarray(k, dtype=np.float32)
    w = np.asarray(random_weights, dtype=np.float32)
    # [B,H,S,D] -> [B*H, D, S]
    qT = np.ascontiguousarray(q.transpose(0, 1, 3, 2)).reshape(B * H, D, S)
    kT = np.ascontiguousarray(k.transpose(0, 1, 3, 2)).reshape(B * H, D, S)
    wt = np.concatenate([w.T, w.T], axis=0)  # [128, D] duplicated halves
    wthi, wtlo = _split_bf16(np.ascontiguousarray(wt))
    qhi, qlo = _split_bf16(qT)
    khi, klo = _split_bf16(kT)
    in_maps = []
    for c in range(NCORES):
        sel = slice(HPC * c, HPC * (c + 1))
        in_maps.append({
            "qthi": np.ascontiguousarray(qhi[sel]).reshape(PAIRS * 128, S),
            "qtlo": np.ascontiguousarray(qlo[sel]).reshape(PAIRS * 128, S),
            "kthi": np.ascontiguousarray(khi[sel]).reshape(PAIRS * 128, S),
            "ktlo": np.ascontiguousarray(klo[sel]).reshape(PAIRS * 128, S),
            "wthi": wthi,
            "wtlo": wtlo,
        })
    return in_maps


def run_device(q, k, random_weights, cfg=None, trace=None):
    """Compile (cached), run on all 8 cores, return (attn_weights, results)."""
    from concourse.bass_utils import run_bass_kernel_spmd

    cfg = dict(CONFIG if cfg is None else cfg)
    if trace is not None:
        cfg["trace"] = trace
    nc = _get_program(cfg)
    in_maps = make_in_maps(q, k, random_weights)
    res = run_bass_kernel_spmd(
        nc, in_maps, core_ids=list(range(NCORES)), trace=cfg["trace"]
    )
    outs = [res.results[c]["out"].reshape(HPC, S, S) for c in range(NCORES)]
    attn_weights = np.concatenate(outs, axis=0).reshape(B, H, S, S)
    return attn_weights, res


def kernel(q, k, v, random_weights):
    attn_weights, _ = run_device(q, k, random_weights)
    attn_output = np.asarray(v, dtype=np.float32)
    return attn_output, attn_weights


# revision 12
# speedup vs baseline: 1.0266x; 1.0187x over previous
"""Trainium2 Bass kernel for nn_PerformerAttention.

reference math (B,H,S,D = 4,8,2048,64):
    qf = relu(q @ W.T); kf = relu(k @ W.T)          # [B,H,S,D]
    scores = qf @ kf.T                              # [B,H,S,S]
    attn_weights = softmax(scores, axis=-1)
    attn_output  = v * rowsum(attn_weights) == v    # softmax rows sum to 1
    returns (attn_output, attn_weights)

Sharding: B*H = 32 (b,h) pairs, 4 per core across 8 cores.  Host-side
layout prep only: q/k transposed to [.., D, S] (matmul contracts over
the partition dim, so the device needs no transposes) and split into
bf16 hi/lo pairs (hi + lo == fp32 value to ~2^-17).

All matmuls run as 3-term bf16 splits (a@b = ahi@bhi + ahi@blo +
alo@bhi; products are exact in the PE, only the lo@lo term is dropped,
~1e-4..1e-3 rel err) because fp32 matmul costs 2 half-rate passes on
TRN2 (~2x the cost model).  Scores per 128-row tile accumulate in a
[128, S] fp32 PSUM tile; softmax is:
    VectorE: fused PSUM->SBUF copy(-scores) + row-max accumulator
    ScalarE: exp(scale*x+bias) with fused row-sum
    normalize-mul on V or S per cfg pattern; DMA out.
Feature transforms for the next head are emitted one 512-col chunk at
a time between score tiles so they hide inside the V/S-bound pipeline
slack instead of stalling it.
"""

import os
import numpy as np

B, H, S, D = 4, 8, 2048, 64
NCORES = 8
HPC = (B * H) // NCORES      # heads per core = 4
PAIRS = HPC // 2             # head pairs per core = 2
RT = S // 128                # 128-row score tiles per head = 16
NCH = S // 512               # 512-col matmul chunks per row tile = 4

# Tunables (test.py may override before calling kernel()).
CONFIG = {
    "copymax": True,      # fused PSUM->SBUF copy + row-max on VectorE
    "mul_pattern": "vvs", # normalize-mul engine per tile, cycled: v/s/g
    "hiprio": True,       # schedule PSUM-releasing ops ahead of lagging muls
    "trace": False,       # request NTFF profile from the run
}

_CACHE = {}


def _build_program(cfg):
    from contextlib import ExitStack, nullcontext

    import concourse.bacc as bacc
    import concourse.mybir as mybir
    import concourse.tile as tile

    f32 = mybir.dt.float32
    bf16 = mybir.dt.bfloat16
    AF = mybir.ActivationFunctionType
    OP = mybir.AluOpType
    AX = mybir.AxisListType

    nc = bacc.Bacc(
        "TRN2",
        target_bir_lowering=False,
        debug=False,
        num_devices=NCORES,
    )

    ins = {}
    for nm in ("qthi", "qtlo", "kthi", "ktlo"):
        ins[nm] = nc.dram_tensor(nm, [PAIRS * 128, S], bf16, kind="ExternalInput").ap()
    for nm in ("wthi", "wtlo"):
        ins[nm] = nc.dram_tensor(nm, [128, D], bf16, kind="ExternalInput").ap()
    out = nc.dram_tensor("out", [HPC * S, S], f32, kind="ExternalOutput").ap()

    with tile.TileContext(nc) as tc, ExitStack() as ctx:
        const = ctx.enter_context(tc.tile_pool(name="const", bufs=1))
        inp = ctx.enter_context(tc.tile_pool(name="inp", bufs=2))
        feat = ctx.enter_context(tc.tile_pool(name="feat", bufs=2))
        psum = ctx.enter_context(tc.tile_pool(name="psum", bufs=2, space="PSUM"))
        work = ctx.enter_context(tc.tile_pool(name="work", bufs=5))
        stat = ctx.enter_context(tc.tile_pool(name="stat", bufs=8))
        outp = ctx.enter_context(tc.tile_pool(name="outp", bufs=5))

        wh = const.tile([128, D], bf16, tag="wh", name="wh")
        nc.sync.dma_start(wh[:], ins["wthi"][:, :])
        wl = const.tile([128, D], bf16, tag="wl", name="wl")
        nc.sync.dma_start(wl[:], ins["wtlo"][:, :])

        tile_idx = 0
        hp = (lambda: tc.high_priority()) if cfg["hiprio"] else (lambda: nullcontext())

        pending = []  # deferred (rowsum, expt, h, m) normalize+store work

        def flush_tail():
            """Emit recip+mul+DMA for the oldest pending tile.  Deferred one
            tile so the VectorE stream never blocks on ScalarE's row-sum."""
            nonlocal tile_idx
            if not pending:
                return
            rowsum, expt, h, m = pending.pop(0)
            rinv = stat.tile([128, 1], f32, tag="rinv", name="rinv")
            nc.vector.reciprocal(rinv[:], rowsum[:])
            ot = outp.tile([128, S], f32, tag="ot", name="ot")
            eng = cfg["mul_pattern"][tile_idx % len(cfg["mul_pattern"])]
            if eng == "s":
                nc.scalar.activation(ot[:], expt[:], AF.Copy, bias=0.0, scale=rinv[:])
            else:
                nc.vector.tensor_scalar(ot[:], expt[:], rinv[:], None, OP.mult, OP.bypass)
            nc.sync.dma_start(out[h * S + 128 * m : h * S + 128 * (m + 1), :], ot[:])
            tile_idx += 1

        def softmax_tail(ps, h, m):
            """PSUM scores tile -> exp'd SBUF tile (normalize deferred)."""
            negmax = stat.tile([128, 1], f32, tag="negmax", name="negmax")
            rowsum = stat.tile([128, 1], f32, tag="rowsum", name="rowsum")
            expt = work.tile([128, S], f32, tag="expt", name="expt")
            if cfg["copymax"]:
                # sc = -scores (SBUF copy), negmax = min(-scores) = -rowmax.
                # Frees the PSUM tile after this single VectorE pass.  High
                # priority: prefer these over earlier tiles' normalize-muls
                # or the PSUM slots starve the PE.
                sc = work.tile([128, S], f32, tag="sc", name="sc")
                with hp():
                    nc.vector.tensor_scalar(
                        sc[:], ps[:], -1.0, None, OP.mult, OP.min, accum_out=negmax[:]
                    )
                    nc.scalar.activation(
                        expt[:], sc[:], AF.Exp,
                        bias=negmax[:], scale=-1.0, accum_out=rowsum[:],
                    )
            else:
                with hp():
                    nc.vector.reduce_max(negmax[:], ps[:], AX.X, negate=True)
                    nc.scalar.activation(
                        expt[:], ps[:], AF.Exp,
                        bias=negmax[:], scale=1.0, accum_out=rowsum[:],
                    )
            pending.append((rowsum, expt, h, m))
            if len(pending) > 1:
                flush_tail()

        # ---- inputs + features ------------------------------------------
        pair_tiles = {}

        def load_pair(p):
            t = {}
            for nm in ("qthi", "qtlo", "kthi", "ktlo"):
                t[nm] = inp.tile([128, S], bf16, tag=nm, name=nm)
                nc.sync.dma_start(t[nm][:], ins[nm][128 * p : 128 * (p + 1), :])
            pair_tiles[p] = t

        feats = {}  # h -> {"q": (hi, lo), "k": (hi, lo)}

        def alloc_feats(h):
            st = {}
            for w_ in ("q", "k"):
                hi = feat.tile([128, S], bf16, tag=w_ + "hi", name=w_ + "hi")
                lo = feat.tile([128, S], bf16, tag=w_ + "lo", name=w_ + "lo")
                st[w_] = (hi, lo)
            feats[h] = st

        def emit_feature_chunk(h, which, j):
            """One 512-col chunk of the feature transform for head h:
            relu(W^T.T @ xT) into bf16 hi/lo, duplicated across both
            partition halves via PE column tiling."""
            p, e = h // 2, h % 2
            rb = 64 * e
            src_hi = pair_tiles[p]["qthi" if which == "q" else "kthi"]
            src_lo = pair_tiles[p]["qtlo" if which == "q" else "ktlo"]
            hi, lo = feats[h][which]
            cs = slice(512 * j, 512 * (j + 1))
            pf = psum.tile([128, 512], f32, tag="ps", name="pf")
            for c in (0, 64):
                for t, (wmat, smat) in enumerate(
                    ((wh, src_hi), (wh, src_lo), (wl, src_hi))
                ):
                    nc.tensor.matmul(
                        pf[c : c + 64, :],
                        lhsT=wmat[rb : rb + 64, :],
                        rhs=smat[rb : rb + 64, cs],
                        start=(t == 0), stop=(t == 2),
                        tile_position=(rb, c),
                    )
            nc.scalar.activation(hi[:, cs], pf[:, :], AF.Relu)
            # lo = relu(pf) - hi, rounded to bf16 (the dropped residual).
            nc.vector.scalar_tensor_tensor(
                lo[:, cs], pf[:, :], 0.0, hi[:, cs], OP.max, OP.subtract
            )
            del pf

        def score_tile(h, m):
            # The K=64 matmul streams at half rate (1 col / 1.2GHz cycle);
            # row-packing two matmuls on opposite PE halves restores full
            # rate.  Pack the tile's left column half (rows 0-63) against
            # its right half (rows 64-127) — features are duplicated across
            # both partition halves, and both write the same PSUM tile, so
            # serial-tile PSUM pipelining is preserved.
            qhi, qlo = feats[h]["q"]
            khi, klo = feats[h]["k"]
            mc = slice(128 * m, 128 * (m + 1))
            ps = psum.tile([128, S], f32, tag="ps", name="ps")
            for jj in range(NCH // 2):
                for t, (ql, kl) in enumerate(((qhi, khi), (qhi, klo), (qlo, khi))):
                    for half, rb in ((0, 0), (1, 64)):
                        j = jj + half * (NCH // 2)
                        cs = slice(512 * j, 512 * (j + 1))
                        nc.tensor.matmul(
                            ps[:, cs],
                            lhsT=ql[rb : rb + 64, mc],
                            rhs=kl[rb : rb + 64, cs],
                            start=(t == 0), stop=(t == 2),
                            tile_position=(rb, 0),
                        )
            return ps

        # ---- main schedule ----------------------------------------------
        load_pair(0)
        alloc_feats(0)
        for j in range(NCH):
            for w_ in ("q", "k"):
                emit_feature_chunk(0, w_, j)

        for h in range(HPC):
            nxt = h + 1
            # prefetch plan: tile index -> closure
            plan = {}
            if nxt < HPC:
                if nxt % 2 == 0:
                    plan[0] = lambda p=nxt // 2: load_pair(p)
                plan[1] = lambda n=nxt: alloc_feats(n)
                k_ = 0
                for w_ in ("q", "k"):
                    for j in range(NCH):
                        plan[2 + 2 * k_ - (k_ // 7)] = (
                            lambda n=nxt, ww=w_, jj=j: emit_feature_chunk(n, ww, jj)
                        )
                        k_ += 1
            for m in range(RT):
                ps = score_tile(h, m)
                softmax_tail(ps, h, m)
                act = plan.get(m)
                if act:
                    act()
        while pending:
            flush_tail()

    nc.compile()
    return nc


def _cfg_key(cfg):
    return (cfg["copymax"], cfg["mul_pattern"], cfg["hiprio"])


def _get_program(cfg):
    key = _cfg_key(cfg)
    if key not in _CACHE:
        _CACHE[key] = _build_program(cfg)
    return _CACHE[key]


def _split_bf16(x):
    import ml_dtypes

    hi = x.astype(ml_dtypes.bfloat16)
    lo = (x - hi.astype(np.float32)).astype(ml_dtypes.bfloat16)
    return hi, lo


def make_in_maps(q, k, random_weights):
    """Host-side sharding/layout prep -> per-core input dicts."""
    q = np.asarray(q, dtype=np.float32)
    k = np.asarray(k, dtype=np.float32)
    w = np.asarray(random_weights, dtype=np.float32)
    # [B,H,S,D] -> [B*H, D, S]
    qT = np.ascontiguousarray(q.transpose(0, 1, 3, 2)).reshape(B * H, D, S)
    kT = np.ascontiguousarray(k.transpose(0, 1, 3, 2)).reshape(B * H, D, S)
    wt = np.concatenate([w.T, w.T], axis=0)  # [128, D] duplicated halves
    wthi, wtlo = _split_bf16(np.ascontiguousarray(wt))
    qhi, qlo = _split_bf16(qT)
    khi, klo = _split_bf16(kT)
    in_maps = []
    for c in range(NCORES):
        sel = slice(HPC * c, HPC * (c + 1))
        in_maps.append({
            "qthi": np.ascontiguousarray(qhi[sel]).reshape(PAIRS * 128, S),
            "qtlo": np.ascontiguousarray(qlo[sel]).reshape(PAIRS * 128, S),
            "kthi": np.ascontiguousarray(khi[sel]).reshape(PAIRS * 128, S),
            "ktlo": np.ascontiguousarray(klo[sel]).reshape(PAIRS * 128, S),
            "wthi": wthi,
            "wtlo": wtlo,
        })
    return in_maps


def run_device(q, k, random_weights, cfg=None, trace=None):
    """Compile (cached), run on all 8 cores, return (attn_weights, results)."""
    from concourse.bass_utils import run_bass_kernel_spmd

    cfg = dict(CONFIG if cfg is None else cfg)
    if trace is not None:
        cfg["trace"] = trace
    nc = _get_program(cfg)
    in_maps = make_in_maps(q, k, random_weights)
    res = run_bass_kernel_spmd(
        nc, in_maps, core_ids=list(range(NCORES)), trace=cfg["trace"]
    )
    outs = [res.results[c]["out"].reshape(HPC, S, S) for c in range(NCORES)]
    attn_weights = np.concatenate(outs, axis=0).reshape(B, H, S, S)
    return attn_weights, res


def kernel(q, k, v, random_weights):
    attn_weights, _ = run_device(q, k, random_weights)
    attn_output = np.asarray(v, dtype=np.float32)
    return attn_output, attn_weights


# revision 13
# speedup vs baseline: 1.0577x; 1.0303x over previous
"""Trainium2 Bass kernel for nn_PerformerAttention.

reference math (B,H,S,D = 4,8,2048,64):
    qf = relu(q @ W.T); kf = relu(k @ W.T)          # [B,H,S,D]
    scores = qf @ kf.T                              # [B,H,S,S]
    attn_weights = softmax(scores, axis=-1)
    attn_output  = v * rowsum(attn_weights) == v    # softmax rows sum to 1
    returns (attn_output, attn_weights)

Sharding: B*H = 32 (b,h) pairs, 4 per core across 8 cores.  Host-side
layout prep only: q/k transposed to [.., D, S] (matmul contracts over
the partition dim, so the device needs no transposes) and split into
bf16 hi/lo pairs (hi + lo == fp32 value to ~2^-17).

All matmuls run as 3-term bf16 splits (a@b = ahi@bhi + ahi@blo +
alo@bhi; products are exact in the PE, only the lo@lo term is dropped,
~1e-4..1e-3 rel err) because fp32 matmul costs 2 half-rate passes on
TRN2 (~2x the cost model).  Scores per 128-row tile accumulate in a
[128, S] fp32 PSUM tile; softmax is:
    VectorE: fused PSUM->SBUF copy(-scores) + row-max accumulator
    ScalarE: exp(scale*x+bias) with fused row-sum
    normalize-mul on V or S per cfg pattern; DMA out.
Feature transforms for the next head are emitted one 512-col chunk at
a time between score tiles so they hide inside the V/S-bound pipeline
slack instead of stalling it.
"""

import os
import numpy as np

B, H, S, D = 4, 8, 2048, 64
NCORES = 8
HPC = (B * H) // NCORES      # heads per core = 4
PAIRS = HPC // 2             # head pairs per core = 2
RT = S // 128                # 128-row score tiles per head = 16
NCH = S // 512               # 512-col matmul chunks per row tile = 4

# Tunables (test.py may override before calling kernel()).
CONFIG = {
    "copymax": True,      # fused PSUM->SBUF copy + row-max on VectorE
    "mul_pattern": "vvs", # normalize-mul engine per tile, cycled: v/s/g
    "hiprio": True,       # schedule PSUM-releasing ops ahead of lagging muls
    "bf16_out": False,    # bf16 exp tile + 4x DVE mul + SWDGE cast-DMA out
    "trace": False,       # request NTFF profile from the run
}

_CACHE = {}


def _build_program(cfg):
    from contextlib import ExitStack, nullcontext

    import concourse.bacc as bacc
    import concourse.mybir as mybir
    import concourse.tile as tile

    f32 = mybir.dt.float32
    bf16 = mybir.dt.bfloat16
    AF = mybir.ActivationFunctionType
    OP = mybir.AluOpType
    AX = mybir.AxisListType

    nc = bacc.Bacc(
        "TRN2",
        target_bir_lowering=False,
        debug=False,
        num_devices=NCORES,
    )

    ins = {}
    for nm in ("qthi", "qtlo", "kthi", "ktlo"):
        ins[nm] = nc.dram_tensor(nm, [PAIRS * 128, S], bf16, kind="ExternalInput").ap()
    for nm in ("wthi", "wtlo"):
        ins[nm] = nc.dram_tensor(nm, [128, D], bf16, kind="ExternalInput").ap()
    out = nc.dram_tensor("out", [HPC * S, S], f32, kind="ExternalOutput").ap()

    with tile.TileContext(nc) as tc, ExitStack() as ctx:
        const = ctx.enter_context(tc.tile_pool(name="const", bufs=1))
        inp = ctx.enter_context(tc.tile_pool(name="inp", bufs=2))
        feat = ctx.enter_context(tc.tile_pool(name="feat", bufs=2))
        psum = ctx.enter_context(tc.tile_pool(name="psum", bufs=2, space="PSUM"))
        work = ctx.enter_context(tc.tile_pool(name="work", bufs=5))
        stat = ctx.enter_context(tc.tile_pool(name="stat", bufs=8))
        outp = ctx.enter_context(tc.tile_pool(name="outp", bufs=5))

        wh = const.tile([128, D], bf16, tag="wh", name="wh")
        nc.sync.dma_start(wh[:], ins["wthi"][:, :])
        wl = const.tile([128, D], bf16, tag="wl", name="wl")
        nc.sync.dma_start(wl[:], ins["wtlo"][:, :])

        tile_idx = 0
        hp = (lambda: tc.high_priority()) if cfg["hiprio"] else (lambda: nullcontext())

        pending = []  # deferred (rowsum, expt, h, m) normalize+store work

        def flush_tail():
            """Emit recip+mul+DMA for the oldest pending tile.  Deferred one
            tile so the VectorE stream never blocks on ScalarE's row-sum."""
            nonlocal tile_idx
            if not pending:
                return
            rowsum, expt, h, m = pending.pop(0)
            rinv = stat.tile([128, 1], f32, tag="rinv", name="rinv")
            nc.vector.reciprocal(rinv[:], rowsum[:])
            odt = bf16 if cfg["bf16_out"] else f32
            ot = outp.tile([128, S], odt, tag="ot", name="ot")
            eng = cfg["mul_pattern"][tile_idx % len(cfg["mul_pattern"])]
            if eng == "s" and not cfg["bf16_out"]:
                nc.scalar.activation(ot[:], expt[:], AF.Copy, bias=0.0, scale=rinv[:])
            else:
                nc.vector.tensor_scalar(ot[:], expt[:], rinv[:], None, OP.mult, OP.bypass)
            dst = out[h * S + 128 * m : h * S + 128 * (m + 1), :]
            if cfg["bf16_out"]:
                nc.gpsimd.dma_start(dst, ot[:])  # SWDGE casts bf16 -> f32
            else:
                nc.sync.dma_start(dst, ot[:])
            tile_idx += 1

        def softmax_tail(ps, h, m):
            """PSUM scores tile -> exp'd SBUF tile (normalize deferred)."""
            negmax = stat.tile([128, 1], f32, tag="negmax", name="negmax")
            rowsum = stat.tile([128, 1], f32, tag="rowsum", name="rowsum")
            edt = bf16 if cfg["bf16_out"] else f32
            expt = work.tile([128, S], edt, tag="expt", name="expt")
            if cfg["copymax"]:
                # sc = -scores (SBUF copy), negmax = min(-scores) = -rowmax.
                # Frees the PSUM tile after this single VectorE pass.  High
                # priority: prefer these over earlier tiles' normalize-muls
                # or the PSUM slots starve the PE.
                sc = work.tile([128, S], f32, tag="sc", name="sc")
                with hp():
                    nc.vector.tensor_scalar(
                        sc[:], ps[:], -1.0, None, OP.mult, OP.min, accum_out=negmax[:]
                    )
                    nc.scalar.activation(
                        expt[:], sc[:], AF.Exp,
                        bias=negmax[:], scale=-1.0, accum_out=rowsum[:],
                    )
            else:
                with hp():
                    nc.vector.reduce_max(negmax[:], ps[:], AX.X, negate=True)
                    nc.scalar.activation(
                        expt[:], ps[:], AF.Exp,
                        bias=negmax[:], scale=1.0, accum_out=rowsum[:],
                    )
            pending.append((rowsum, expt, h, m))
            if len(pending) > 1:
                flush_tail()

        # ---- inputs + features ------------------------------------------
        pair_tiles = {}

        def load_pair(p):
            t = {}
            for nm in ("qthi", "qtlo", "kthi", "ktlo"):
                t[nm] = inp.tile([128, S], bf16, tag=nm, name=nm)
                nc.sync.dma_start(t[nm][:], ins[nm][128 * p : 128 * (p + 1), :])
            pair_tiles[p] = t

        feats = {}  # h -> {"q": (hi, lo), "k": (hi, lo)}

        def alloc_feats(h):
            st = {}
            for w_ in ("q", "k"):
                hi = feat.tile([128, S], bf16, tag=w_ + "hi", name=w_ + "hi")
                lo = feat.tile([128, S], bf16, tag=w_ + "lo", name=w_ + "lo")
                st[w_] = (hi, lo)
            feats[h] = st

        def emit_feature_chunk(h, which, j):
            """One 512-col chunk of the feature transform for head h:
            relu(W^T.T @ xT) into bf16 hi/lo, duplicated across both
            partition halves via PE column tiling."""
            p, e = h // 2, h % 2
            rb = 64 * e
            src_hi = pair_tiles[p]["qthi" if which == "q" else "kthi"]
            src_lo = pair_tiles[p]["qtlo" if which == "q" else "ktlo"]
            hi, lo = feats[h][which]
            cs = slice(512 * j, 512 * (j + 1))
            pf = psum.tile([128, 512], f32, tag="ps", name="pf")
            for c in (0, 64):
                for t, (wmat, smat) in enumerate(
                    ((wh, src_hi), (wh, src_lo), (wl, src_hi))
                ):
                    nc.tensor.matmul(
                        pf[c : c + 64, :],
                        lhsT=wmat[rb : rb + 64, :],
                        rhs=smat[rb : rb + 64, cs],
                        start=(t == 0), stop=(t == 2),
                        tile_position=(rb, c),
                    )
            nc.scalar.activation(hi[:, cs], pf[:, :], AF.Relu)
            # lo = relu(pf) - hi, rounded to bf16 (the dropped residual).
            nc.vector.scalar_tensor_tensor(
                lo[:, cs], pf[:, :], 0.0, hi[:, cs], OP.max, OP.subtract
            )
            del pf

        def score_tile(h, m):
            # The K=64 matmul streams at half rate (1 col / 1.2GHz cycle);
            # row-packing two matmuls on opposite PE halves restores full
            # rate.  Pack the tile's left column half (rows 0-63) against
            # its right half (rows 64-127) — features are duplicated across
            # both partition halves, and both write the same PSUM tile, so
            # serial-tile PSUM pipelining is preserved.
            qhi, qlo = feats[h]["q"]
            khi, klo = feats[h]["k"]
            mc = slice(128 * m, 128 * (m + 1))
            ps = psum.tile([128, S], f32, tag="ps", name="ps")
            for jj in range(NCH // 2):
                for t, (ql, kl) in enumerate(((qhi, khi), (qhi, klo), (qlo, khi))):
                    for half, rb in ((0, 0), (1, 64)):
                        j = jj + half * (NCH // 2)
                        cs = slice(512 * j, 512 * (j + 1))
                        nc.tensor.matmul(
                            ps[:, cs],
                            lhsT=ql[rb : rb + 64, mc],
                            rhs=kl[rb : rb + 64, cs],
                            start=(t == 0), stop=(t == 2),
                            tile_position=(rb, 0),
                        )
            return ps

        # ---- main schedule ----------------------------------------------
        load_pair(0)
        alloc_feats(0)
        for j in range(NCH):
            for w_ in ("q", "k"):
                emit_feature_chunk(0, w_, j)

        for h in range(HPC):
            nxt = h + 1
            # prefetch plan: tile index -> closure
            plan = {}
            if nxt < HPC:
                if nxt % 2 == 0:
                    plan[0] = lambda p=nxt // 2: load_pair(p)
                plan[1] = lambda n=nxt: alloc_feats(n)
                k_ = 0
                for w_ in ("q", "k"):
                    for j in range(NCH):
                        plan[2 + 2 * k_ - (k_ // 7)] = (
                            lambda n=nxt, ww=w_, jj=j: emit_feature_chunk(n, ww, jj)
                        )
                        k_ += 1
            for m in range(RT):
                ps = score_tile(h, m)
                softmax_tail(ps, h, m)
                act = plan.get(m)
                if act:
                    act()
        while pending:
            flush_tail()

    nc.compile()
    return nc


def _cfg_key(cfg):
    return (cfg["copymax"], cfg["mul_pattern"], cfg["hiprio"], cfg["bf16_out"])


def _get_program(cfg):
    key = _cfg_key(cfg)
    if key not in _CACHE:
        _CACHE[key] = _build_program(cfg)
    return _CACHE[key]


def _split_bf16(x):
    import ml_dtypes

    hi = x.astype(ml_dtypes.bfloat16)
    lo = (x - hi.astype(np.float32)).astype(ml_dtypes.bfloat16)
    return hi, lo


def make_in_maps(q, k, random_weights):
    """Host-side sharding/layout prep -> per-core input dicts."""
    q = np.asarray(q, dtype=np.float32)
    k = np.asarray(k, dtype=np.float32)
    w = np.asarray(random_weights, dtype=np.float32)
    # [B,H,S,D] -> [B*H, D, S]
    qT = np.ascontiguousarray(q.transpose(0, 1, 3, 2)).reshape(B * H, D, S)
    kT = np.ascontiguousarray(k.transpose(0, 1, 3, 2)).reshape(B * H, D, S)
    wt = np.concatenate([w.T, w.T], axis=0)  # [128, D] duplicated halves
    wthi, wtlo = _split_bf16(np.ascontiguousarray(wt))
    qhi, qlo = _split_bf16(qT)
    khi, klo = _split_bf16(kT)
    in_maps = []
    for c in range(NCORES):
        sel = slice(HPC * c, HPC * (c + 1))
        in_maps.append({
            "qthi": np.ascontiguousarray(qhi[sel]).reshape(PAIRS * 128, S),
            "qtlo": np.ascontiguousarray(qlo[sel]).reshape(PAIRS * 128, S),
            "kthi": np.ascontiguousarray(khi[sel]).reshape(PAIRS * 128, S),
            "ktlo": np.ascontiguousarray(klo[sel]).reshape(PAIRS * 128, S),
            "wthi": wthi,
            "wtlo": wtlo,
        })
    return in_maps


def run_device(q, k, random_weights, cfg=None, trace=None):
    """Compile (cached), run on all 8 cores, return (attn_weights, results)."""
    from concourse.bass_utils import run_bass_kernel_spmd

    cfg = dict(CONFIG if cfg is None else cfg)
    if trace is not None:
        cfg["trace"] = trace
    nc = _get_program(cfg)
    in_maps = make_in_maps(q, k, random_weights)
    res = run_bass_kernel_spmd(
        nc, in_maps, core_ids=list(range(NCORES)), trace=cfg["trace"]
    )
    outs = [res.results[c]["out"].reshape(HPC, S, S) for c in range(NCORES)]
    attn_weights = np.concatenate(outs, axis=0).reshape(B, H, S, S)
    return attn_weights, res


def kernel(q, k, v, random_weights):
    attn_weights, _ = run_device(q, k, random_weights)
    attn_output = np.asarray(v, dtype=np.float32)
    return attn_output, attn_weights
